# revision 13
# baseline (speedup 1.0000x reference)
"""STFT (DFT-as-conv) kernel for Trainium2, 8 NeuronCores.

Problem: x (16, 262144) f32, hann-windowed DFT kernels wsin/wcos
(2048, 1, 2048); reference reflect-pads by 1024, convolves with hop 512
-> returns (real, -imag), each (16, 2048, 513) f32.

Strategy:
  - Data-parallel over batch: 2 batches per core.
  - Hop-block im2col: n_fft = 4*hop, so the frame matrix is 4 shifted
    views of bt[b, cc, jj, m] = xpad[b, 512*m + 128*cc + jj].
  - Spectral symmetry: bins k and 2048-k mirror (cos even, sin odd);
    device computes bins 0..1151, host mirrors the remaining 896.
  - Time-reversal fold: the hann window is symmetric, so
    W[k, 2048-n] = +/- W[k, n]. Device folds frames into
    z+/-[c] = y[n] +/- y[2048-n] (DVE adds on shifted views of bt and a
    host-prepared reversed copy rev4), halving the contraction to 1024.
    win[0] = 0 kills the unpaired n=0 lane; sin(pi*n) = 0 kills the sin
    n=1024 term; the cos n=1024 term is a K=1 rank-1 matmul.
  - fp32r matmuls (full PE rate at even moving-dim >= 256). Frames
    padded 513 -> 514, split 258+256 (PSUM bank caps N at 512).
"""

import sys

sys.path.insert(0, "/opt/trn_rl_repo")

import numpy as np

BATCH = 16
LENGTH = 262144
N_FFT = 2048
HOP = 512
FRAMES = 513          # LENGTH // HOP + 1
PAD_FRAMES = 514      # frames padded to even for fp32r
M_CHUNKS = 9          # bin chunks of 128 computed on device
M_KEEP = M_CHUNKS * 128   # 1152 bins; 896 more mirrored on host
BLOCKS = 516          # padded length 264192 / 512
BT_COLS = 520         # blocks padded so shifted views stay in range
N_GROUPS = ((0, 258), (258, 256))  # frame groups: start, size (even)
CORES = 8
B_PER_CORE = BATCH // CORES
EXT = HOP * BT_COLS + 1537  # zero-extended xpad length for rev4 strides

_cache = {}


def _build_device_kernel():
    import concourse.bacc as bacc
    import concourse.mybir as mybir
    from concourse import tile

    nc = bacc.Bacc("TRN2", target_bir_lowering=False, debug=False,
                   num_devices=CORES)
    f32 = mybir.dt.float32
    f32r = mybir.dt.float32r

    bt_d = nc.dram_tensor("bt", [B_PER_CORE, 4, 128, BT_COLS], f32r,
                          kind="ExternalInput")
    rv_d = nc.dram_tensor("rv", [B_PER_CORE, 4, 128, BT_COLS], f32r,
                          kind="ExternalInput")
    w_d = nc.dram_tensor("w", [2 * M_CHUNKS, 128, 8, 128], f32r,
                         kind="ExternalInput")
    o_d = nc.dram_tensor("o", [B_PER_CORE, 2 * M_CHUNKS, 128, PAD_FRAMES],
                         f32, kind="ExternalOutput")

    with tile.TileContext(nc) as tc:
        with (
            tc.tile_pool(name="btp", bufs=1) as btp,
            tc.tile_pool(name="zp", bufs=1) as zpool,
            tc.tile_pool(name="wp", bufs=4) as wp,
            tc.tile_pool(name="op", bufs=4) as op,
            tc.tile_pool(name="psp", bufs=8, space="PSUM") as psp,
        ):
            # Column-split input DMAs and folds so the first frame-group's
            # matmuls only wait on ~1/4 of the input: DMA half H0 fills
            # cols [0, 264), H1 fills [264, 520); fold half A covers z
            # cols [0, 258) (reads input cols <= 260), B covers [258, 514).
            SPLIT = 264
            bts = [[None] * 4 for _ in range(B_PER_CORE)]
            rvs = [[None] * 4 for _ in range(B_PER_CORE)]
            zt = [[[None] * 8 for _ in range(B_PER_CORE)] for _ in range(2)]
            for b in range(B_PER_CORE):
                for c in range(4):
                    bts[b][c] = btp.tile([128, BT_COLS], f32r,
                                         name=f"bt{b}{c}", tag=f"bt{b}{c}")
                    rvs[b][c] = btp.tile([128, BT_COLS], f32r,
                                         name=f"rv{b}{c}", tag=f"rv{b}{c}")
                for s in range(2):
                    for c in range(8):
                        zt[s][b][c] = zpool.tile(
                            [128, PAD_FRAMES], f32r,
                            name=f"z{s}{b}{c}", tag=f"z{s}{b}{c}")

            def emit_dma_half(b, lo, hi):
                for c in range(4):
                    nc.sync.dma_start(out=bts[b][c][:, lo:hi],
                                      in_=bt_d[b, c, :, lo:hi])
                    nc.sync.dma_start(out=rvs[b][c][:, lo:hi],
                                      in_=rv_d[b, c, :, lo:hi])

            def emit_fold_half(b, lo, hi):
                for s, dve_op in ((0, nc.vector.tensor_add),
                                  (1, nc.vector.tensor_sub)):
                    for c in range(8):
                        sh, rh = c // 4, 1 - c // 4
                        dve_op(out=zt[s][b][c][:, lo:hi],
                               in0=bts[b][c % 4][:, lo + sh:hi + sh],
                               in1=rvs[b][c % 4][:, lo + rh:hi + rh])
                # lane (c=0, jj=0) carries the n=1024 cos term: win[0] = 0
                # frees the n=0 weight slot, so host puts wcos[:, 1024]
                # there and z+ lane 0 must hold y_f[1024].
                nc.vector.tensor_copy(
                    out=zt[0][b][0][0:1, lo:hi],
                    in_=bts[b][0][0:1, lo + 2:hi + 2])

            MID = N_GROUPS[1][0]
            emit_dma_half(0, 0, SPLIT)
            emit_fold_half(0, 0, MID)
            emit_dma_half(0, SPLIT, BT_COLS)
            emit_dma_half(1, 0, SPLIT)
            emit_fold_half(0, MID, PAD_FRAMES)
            emit_fold_half(1, 0, MID)
            emit_dma_half(1, SPLIT, BT_COLS)
            emit_fold_half(1, MID, PAD_FRAMES)

            for u in range(2 * M_CHUNKS):
                kern, mc = divmod(u, M_CHUNKS)
                wt = wp.tile([128, 8, 128], f32r)
                nc.sync.dma_start(out=wt, in_=w_d[u])
                for b in range(B_PER_CORE):
                    ot = op.tile([128, PAD_FRAMES], f32)
                    for f0, ng in N_GROUPS:
                        ps = psp.tile([128, ng], f32)
                        for c in range(8):
                            nc.tensor.matmul(
                                ps, wt[:, c, :],
                                zt[kern][b][c][:, f0:f0 + ng],
                                start=(c == 0), stop=(c == 7))
                        nc.vector.tensor_copy(out=ot[:, f0:f0 + ng], in_=ps)
                    nc.sync.dma_start(out=o_d[b, u], in_=ot)
    nc.compile()
    return nc


def _get_nc():
    if "nc" not in _cache:
        _cache["nc"] = _build_device_kernel()
    return _cache["nc"]


def _host_prep(x, wsin, wcos):
    x = np.asarray(x, dtype=np.float32)
    wsin = np.asarray(wsin, dtype=np.float32).reshape(N_FFT, N_FFT)
    wcos = np.asarray(wcos, dtype=np.float32).reshape(N_FFT, N_FFT)

    xpad = np.pad(x, ((0, 0), (N_FFT // 2, N_FFT // 2)), mode="reflect")
    plen = xpad.shape[1]
    bt = np.zeros((BATCH, 4, 128, BT_COLS), np.float32)
    bt[:, :, :, :BLOCKS] = xpad.reshape(BATCH, BLOCKS, 4, 128) \
                               .transpose(0, 2, 3, 1)

    # rev4[b, cc, jj, m] = xe[512m + 1536 - 128cc - jj] (zero-extended)
    xe = np.zeros((BATCH, EXT), np.float32)
    xe[:, :plen] = xpad
    swv = np.lib.stride_tricks.sliding_window_view(xe, 512, axis=1)
    q = swv[:, 1025::HOP, :][:, :BT_COLS]      # [b, m, q] = xe[512m+1025+q]
    rev4 = np.ascontiguousarray(
        q[:, :, ::-1].transpose(0, 2, 1)).reshape(BATCH, 4, 128, BT_COLS)

    # folded weights wf[kern*9+mc, jj, c, mm] = wm[128mc+mm, 128c+jj];
    # minus folded into the sin kernel (reference returns -imag).
    wf = np.empty((2 * M_CHUNKS, 128, 8, 128), np.float32)
    for kern, wm in enumerate((wcos, -wsin)):
        wk = np.ascontiguousarray(wm[:M_KEEP, :1024].T)  # (1024, 1152)
        wf[kern * M_CHUNKS:(kern + 1) * M_CHUNKS] = (
            wk.reshape(8, 128, M_CHUNKS, 128).transpose(2, 1, 0, 3))
    # n=0 lane is dead (win[0] = 0); reuse it for the cos n=1024 column
    # (z+ chunk-0 lane 0 is patched to y_f[1024] on device).
    wf[:M_CHUNKS, 0, 0, :] = wcos[:M_KEEP, 1024].reshape(M_CHUNKS, 128)
    return bt, rev4, wf


def _host_assemble(outs):
    # outs: list of 8 arrays (B_PER_CORE, 18, 128, 514)
    o = np.concatenate(outs, axis=0)[..., :FRAMES]  # (16, 18, 128, 513)
    o = o.reshape(BATCH, 2, M_KEEP, FRAMES)
    real_h, d_h = o[:, 0], o[:, 1]  # d = -imag
    mirror = slice(N_FFT - M_KEEP, 0, -1)  # bins 896..1
    real = np.concatenate([real_h, real_h[:, mirror]], axis=1)
    nimag = np.concatenate([d_h, -d_h[:, mirror]], axis=1)
    return (np.ascontiguousarray(real, dtype=np.float32),
            np.ascontiguousarray(nimag, dtype=np.float32))


def kernel(x, wsin, wcos):
    from concourse.bass_utils import run_bass_kernel_spmd

    nc = _get_nc()
    bt, rev4, wf = _host_prep(x, wsin, wcos)
    in_maps = [
        {"bt": bt[i * B_PER_CORE:(i + 1) * B_PER_CORE],
         "rv": rev4[i * B_PER_CORE:(i + 1) * B_PER_CORE],
         "w": wf}
        for i in range(CORES)
    ]
    res = run_bass_kernel_spmd(nc, in_maps, core_ids=list(range(CORES)))
    return _host_assemble([res.results[i]["o"] for i in range(CORES)])
